# revision 49
# baseline (speedup 1.0000x reference)
# Trainium2 Bass kernel for BidirectionalCrossAttentionBlock.
#
# Key algebraic facts used (validated vs the reference to 1e-5):
#  * attn_i2t is a softmax over a size-1 axis -> identically 1.0, so
#    img_guided[b,c,n] = v_txt[b,c] (broadcast).
#  * The t2i attention, gating and out-projection collapse to rank-2
#    updates over the image:
#      out1 = x + wv (x) alpha + wt (x) beta + b_out
#    with per-position alpha[n] = img_imp/denom, beta[n] = txt_imp/denom,
#    wv = w_out@v_txt, wt = w_out@(w_img_v@s),
#    s[c] = g[c]*(sum_n u[n] x[c,n] - K1)/Z + b[c],
#    u[n] = exp(SCALE*logit[n])*rstd[n], Z = sum exp, K1 = sum u*m.
#  * logit[n] and the image gate are single matvecs against raw x plus
#    per-position corrections from the LN stats (m[n], rstd[n]).
#  * Only real heavy compute: the two FFN 1x1 convs; done as fp32r
#    matmuls (full PE rate at N=512) with the rank-2/bias terms folded
#    in as small extra-K matmuls.
#
# Sharding: pure data parallel, 2 batches per core on 8 cores.
#
# All parameters/constants are packed into a single "wblob" input tensor
# (sliced on-device with explicit access patterns): per-call argument
# handling through the PJRT tunnel costs ~11us per argument, so 2 args
# instead of 26 saves ~0.25ms per call.

import os
import numpy as np

import concourse.bacc as bacc
import concourse.tile as tile
import concourse.mybir as mybir
from concourse import bass_utils
from concourse.masks import make_identity

F32 = mybir.dt.float32
F32R = mybir.dt.float32r
AL = mybir.AluOpType
AF = mybir.ActivationFunctionType

B, C, H, W = 16, 256, 64, 64
GUIDE = 512
HW = 64 * 64  # 4096
NCORES = 8
BSH = B // NCORES  # 2 batches per core
SCALE = (C // 4) ** -0.5  # heads=4 -> 0.125
EPS_LN = 1e-5
EPS_FUSE = 1e-8
NBLK = 8          # 4096 / 512
BLK = 512
CT = 2            # channel tiles of 128
KT = GUIDE // 128  # 4

# ---- packed weight-blob layout (host packs, device slices by offset) ----
WSPEC = [
    ('wf1T', (C, C)), ('wf2T', (C, C)), ('woutT', (C, C)),
    ('wivT', (C, C)), ('wik', (C, C)),
    ('wtqT', (GUIDE, C)), ('wtvT', (GUIDE, C)),
    ('g2d', (128, CT)), ('bln2d', (128, CT)), ('bb2d', (128, CT)),
    ('cc12d', (128, CT)), ('wg2d', (128, CT)),
    ('gt2d', (1, GUIDE)), ('bt2d', (1, GUIDE)), ('wtg2d', (1, GUIDE)),
    ('btg2d', (2, 1)), ('misc', (1, 8)),
    ('conesr', (1, 128)), ('id128d', (128, 128)),
    ('bind64', (64, 2)),
    ('txt', (BSH, GUIDE)),
]
OFFS = {}
_off = 0
for _n, _s in WSPEC:
    OFFS[_n] = _off
    _off += int(np.prod(_s))
WTOT = _off

_CACHE = {}


def _build(debug=False, repeat=None):
    nc = bacc.Bacc("TRN2", target_bir_lowering=False, debug=False)

    img = nc.dram_tensor("img", [BSH, C, HW], F32R, kind="ExternalInput")
    wblob = nc.dram_tensor("wblob", [1, WTOT], F32, kind="ExternalInput")
    yout = nc.dram_tensor("yout", [BSH, C, HW], F32, kind="ExternalOutput")
    if debug:
        dbg_ext = nc.dram_tensor("dbg_ext", [BSH, 4, HW], F32, kind="ExternalOutput")
        dbg_s = nc.dram_tensor("dbg_s", [BSH, 128, CT], F32, kind="ExternalOutput")
        dbg_sc = nc.dram_tensor("dbg_sc", [BSH, 1, 8], F32, kind="ExternalOutput")

    env = {k: v for k, v in locals().items()}
    from contextlib import ExitStack
    with tile.TileContext(nc) as tc, ExitStack() as ctx:
        env["wp"] = ctx.enter_context(tc.tile_pool(name="wp", bufs=1))
        env["xp"] = ctx.enter_context(tc.tile_pool(name="xp", bufs=2))
        env["bigp"] = ctx.enter_context(tc.tile_pool(name="bigp", bufs=2))
        env["outp"] = ctx.enter_context(tc.tile_pool(name="outp", bufs=2))
        env["smp"] = ctx.enter_context(tc.tile_pool(name="smp", bufs=2))
        env["drp"] = ctx.enter_context(tc.tile_pool(name="drp", bufs=2, space="DRAM"))
        env["psC"] = ctx.enter_context(tc.tile_pool(name="psC", bufs=3, space="PSUM"))
        env["psA"] = ctx.enter_context(tc.tile_pool(name="psA", bufs=2, space="PSUM"))
        env["psB"] = ctx.enter_context(tc.tile_pool(name="psB", bufs=2, space="PSUM"))
        env["psM"] = ctx.enter_context(tc.tile_pool(name="psM", bufs=1, space="PSUM"))
        _emit(nc, tc, env, debug, repeat)
    nc.compile()
    return nc


def _emit(nc, tc, env, debug, repeat=None):
    STAGE = float(os.environ.get("KSTAGE", "9"))
    REPEAT = int(os.environ.get("KREPEAT", "1")) if repeat is None else repeat
    wp, xp, bigp, outp, smp, drp = (env[k] for k in ("wp", "xp", "bigp", "outp", "smp", "drp"))
    psC, psA, psB, psM = (env[k] for k in ("psC", "psA", "psB", "psM"))
    img, yout = env["img"], env["yout"]

    base = env["wblob"].ap()

    def wap(name, dims, off=0, dt_=None):
        # dims: list of [stride, count]; offsets/strides in elements
        a = base.__class__(tensor=base.tensor, offset=OFFS[name] + off,
                           ap=[list(d) for d in dims])
        if dt_ is not None:
            a = a.bitcast(dt_)
        return a

    def body():
        # ---------------- load weights/consts from blob ----------------
        # Consolidated: the blob layout is contiguous, so the five [C,C]
        # fp32r matrices land in ONE DMA as [128, 10, C] (128-row chunks),
        # the two [GUIDE,C] in one [128, 8, C], the five [128,CT] vectors in
        # one [128, 5, CT], the three [1,GUIDE] rows in one broadcast
        # [2, 3, GUIDE]. Each dma_start costs ~565ns of SP sequencer time,
        # so ~30 -> ~11 descriptors-config saves ~11us at the head.
        W5 = wp.tile([128, 10, C], F32R, tag="W5")
        nc.sync.dma_start(out=W5, in_=wap("wf1T", [[C, 128], [128 * C, 10], [1, C]],
                                          dt_=F32R))
        wf1, wf2 = W5[:, 0:2, :], W5[:, 2:4, :]
        wou, wiv, wik_s = W5[:, 4:6, :], W5[:, 6:8, :], W5[:, 8:10, :]
        W8 = wp.tile([128, 8, C], F32R, tag="W8")
        nc.sync.dma_start(out=W8, in_=wap("wtqT", [[C, 128], [128 * C, 8], [1, C]],
                                          dt_=F32R))
        wtq, wtv = W8[:, 0:4, :], W8[:, 4:8, :]

        V5 = wp.tile([128, 5, CT], F32, tag="V5")
        nc.sync.dma_start(out=V5, in_=wap("g2d", [[CT, 128], [128 * CT, 5], [1, CT]]))
        g2, bln2, bb2, cc12, wg2 = (V5[:, i, :] for i in range(5))

        R3 = wp.tile([2, 3, GUIDE], F32, tag="R3")
        nc.sync.dma_start(out=R3, in_=wap("gt2d", [[0, 2], [GUIDE, 3], [1, GUIDE]]))
        gt2, bt2, wtg2 = R3[:, 0, :], R3[:, 1, :], R3[:, 2, :]
        btg2 = wp.tile([2, 1], F32, tag="btg2d")
        nc.sync.dma_start(out=btg2, in_=wap("btg2d", [[1, 2], [1, 1]]))
        misc_sb = wp.tile([1, 8], F32, tag="misc")
        nc.sync.dma_start(out=misc_sb, in_=wap("misc", [[8, 1], [1, 8]]))

        onesrow = wp.tile([1, 128], F32R, tag="onesrow")
        nc.sync.dma_start(out=onesrow, in_=wap("conesr", [[128, 1], [1, 128]], dt_=F32R))
        onescol = wp.tile([128, 1], F32R, tag="onescol")
        nc.sync.dma_start(out=onescol, in_=wap("conesr", [[1, 128], [1, 1]], dt_=F32R))
        ones64x = wp.tile([64, 128], F32R, tag="ones64x")
        nc.sync.dma_start(out=ones64x, in_=wap("conesr", [[0, 64], [1, 128]], dt_=F32R))
        id128 = wp.tile([128, 128], F32R, tag="id128")
        nc.sync.dma_start(out=id128, in_=wap("id128d", [[128, 128], [1, 128]], dt_=F32R))
        bind64 = wp.tile([64, 2], F32, tag="bind64")
        nc.sync.dma_start(out=bind64, in_=wap("bind64", [[2, 64], [1, 2]]))
        id2 = wp.tile([2, 2], F32, tag="id2")
        make_identity(nc, id2[:])
        eps2 = wp.tile([2, 1], F32, tag="eps2")
        nc.vector.memset(eps2[:], EPS_LN)
        eps64 = wp.tile([64, 1], F32, tag="eps64")
        nc.vector.memset(eps64[:], EPS_LN)

        if STAGE <= 0:
            return
        # txt load first (head of the preamble dependency chain), then
        # prefetch img tiles for both batches so the bulk input DMA overlaps
        # the txt-side preamble compute
        txt_sb = smp.tile([2, GUIDE], F32, tag="txt", bufs=1)
        nc.sync.dma_start(out=txt_sb, in_=wap("txt", [[GUIDE, 2], [1, GUIDE]]))
        xbs = []
        for b in range(BSH):
            xb_ = []
            for ct in range(CT):
                t = xp.tile([128, HW], F32R, tag=f"x{b}_{ct}", name=f"x{b}_{ct}", bufs=1)
                nc.sync.dma_start(out=t, in_=img.ap()[b, ct * 128:(ct + 1) * 128, :])
                xb_.append(t)
            xbs.append(xb_)
        nt64 = smp.tile([64, 4, 128], F32, tag="nt64", name="nt64", bufs=1)
        bnst = smp.tile([2, 6], F32, tag="bnst")
        nc.vector.bn_stats(out=bnst[:], in_=txt_sb[:])
        mv = smp.tile([2, 2], F32, tag="mv")
        nc.vector.bn_aggr(out=mv[:], in_=bnst[:])
        tstd = smp.tile([2, 1], F32, tag="tstd")
        nc.scalar.activation(tstd[:], mv[:, 1:2], AF.Sqrt, bias=eps2[:], scale=1.0)
        trstd = smp.tile([2, 1], F32, tag="trstd")
        nc.vector.reciprocal(trstd[:], tstd[:])
        txtn = smp.tile([2, GUIDE], F32, tag="txtn", bufs=1)
        nc.vector.tensor_scalar(out=txtn[:], in0=txt_sb[:], scalar1=mv[:, 0:1],
                                scalar2=trstd[:], op0=AL.subtract, op1=AL.mult)
        nc.vector.tensor_mul(txtn[:], txtn[:], gt2[:])
        nc.vector.tensor_add(txtn[:], txtn[:], bt2[:])
        if STAGE <= 0.2:
            return
        # txt_imp = sigmoid(txtn @ w_tgate + b_tgate)  [2,1]
        scr2 = smp.tile([2, GUIDE], F32, tag="scr2", bufs=1)
        tip = smp.tile([2, 1], F32, tag="tip")
        nc.vector.tensor_mul(scr2[:], txtn[:], wtg2[:])
        nc.vector.reduce_sum(tip[:], scr2[:], axis=mybir.AxisListType.X)
        nc.vector.tensor_add(tip[:], tip[:], btg2[:])
        # sigmoid via exp: keeps Act on the exp table set (no table reload)
        ti = smp.tile([2, 1], F32, tag="ti")
        tie = smp.tile([2, 1], F32, tag="tie")
        nc.scalar.activation(tie[:], tip[:], AF.Exp, scale=-1.0)
        nc.vector.tensor_scalar_add(tie[:], tie[:], 1.0)
        nc.vector.reciprocal(ti[:], tie[:])
        if STAGE <= 0.4:
            return
        # transpose ti -> [1, 2]
        pst = psM.tile([1, 2], F32, tag="psm")
        nc.tensor.transpose(pst[:], ti[:], id2[:])
        tiT = smp.tile([1, 2], F32, tag="tiT")
        nc.vector.tensor_copy(tiT[:], pst[:])
        if STAGE <= 0.5:
            return
        # txtn -> transposed fp32r [128, 4, 2]
        txtnT = smp.tile([128, KT, 2], F32R, tag="txtnT")
        for kt in range(KT):
            ps2 = psM.tile([128, 2], F32, tag="psm")
            nc.tensor.transpose(ps2[:], txtn[:, kt * 128:(kt + 1) * 128], id2[:])
            nc.vector.tensor_copy(txtnT[:, kt, :], ps2[:])
        if STAGE <= 0.6:
            return
        # q = w_txt_q @ txt_n, v = w_txt_v @ txt_n   -> [128, mt, b] fp32r
        q_sb = smp.tile([128, CT, 2], F32R, tag="q_sb")
        v_sb = smp.tile([128, CT, 2], F32R, tag="v_sb")
        for (wmat, dst) in ((wtq, q_sb), (wtv, v_sb)):
            for mt in range(CT):
                psq = psM.tile([128, 2], F32, tag="psm")
                for kt in range(KT):
                    nc.tensor.matmul(psq[:], wmat[:, kt, mt * 128:(mt + 1) * 128],
                                     txtnT[:, kt, :], start=(kt == 0), stop=(kt == KT - 1))
                nc.vector.tensor_copy(dst[:, mt, :], psq[:])
        if STAGE <= 0.7:
            return
        # qk = w_img_k.T @ q  -> [128, mt, b] (fp32 copy for DVE use)
        qk_sb = smp.tile([128, CT, 2], F32, tag="qk_sb")
        for mt in range(CT):
            psk = psM.tile([128, 2], F32, tag="psm")
            for kt in range(CT):
                nc.tensor.matmul(psk[:], wik_s[:, kt, mt * 128:(mt + 1) * 128],
                                 q_sb[:, kt, :], start=(kt == 0), stop=(kt == CT - 1))
            nc.vector.tensor_copy(qk_sb[:, mt, :], psk[:])
        if STAGE <= 0.8:
            return
        # wv = w_out @ v -> [128, mt, b] fp32r ; A1 = w_ffn1 @ wv
        wv_sb = smp.tile([128, CT, 2], F32R, tag="wv_sb")
        for mt in range(CT):
            psv = psM.tile([128, 2], F32, tag="psm")
            for kt in range(CT):
                nc.tensor.matmul(psv[:], wou[:, kt, mt * 128:(mt + 1) * 128],
                                 v_sb[:, kt, :], start=(kt == 0), stop=(kt == CT - 1))
            nc.vector.tensor_copy(wv_sb[:, mt, :], psv[:])
        a1_sb = smp.tile([128, CT, 2], F32R, tag="a1_sb")
        for mt in range(CT):
            psa = psM.tile([128, 2], F32, tag="psm")
            for kt in range(CT):
                nc.tensor.matmul(psa[:], wf1[:, kt, mt * 128:(mt + 1) * 128],
                                 wv_sb[:, kt, :], start=(kt == 0), stop=(kt == CT - 1))
            nc.vector.tensor_copy(a1_sb[:, mt, :], psa[:])

        # ---------------- per-batch, phase-major ----------------
        def pass1(b, xb):
            st_ = {}
            st_["xb"] = xb
            # W3 lhsT = [ones, qg_b, wg]
            W3 = smp.tile([128, CT, 3], F32R, tag="W3", name="W3")
            for ct in range(CT):
                nc.vector.tensor_copy(W3[:, ct, 0:1], onescol[:])
                nc.vector.tensor_mul(W3[:, ct, 1:2], qk_sb[:, ct, b:b + 1], g2[:, ct:ct + 1])
                nc.vector.tensor_copy(W3[:, ct, 2:3], wg2[:, ct:ct + 1])
            # Sq = sum qg ; Cq = sum qk*b_ln
            ps1 = psM.tile([1, 2], F32, tag="psm", name="ps1")
            for ct in range(CT):
                nc.tensor.matmul(ps1[:], W3[:, ct, 1:2], onescol[:].to_broadcast((128, 2)),
                                 start=(ct == 0), stop=(ct == CT - 1))
            tcq = smp.tile([128, CT], F32R, tag="tcq", name="tcq")
            for ct in range(CT):
                nc.vector.tensor_mul(tcq[:, ct:ct + 1], qk_sb[:, ct, b:b + 1], bln2[:, ct:ct + 1])
            ps1b = psM.tile([1, 2], F32, tag="psm", name="ps1b")
            for ct in range(CT):
                nc.tensor.matmul(ps1b[:], tcq[:, ct:ct + 1], onescol[:].to_broadcast((128, 2)),
                                 start=(ct == 0), stop=(ct == CT - 1))
            scal = smp.tile([1, 8], F32R, tag="scal", name="scal")
            nc.vector.tensor_copy(scal[0:1, 0:1], ps1[:, 0:1])
            nc.vector.tensor_copy(scal[0:1, 1:2], ps1b[:, 0:1])
            nc.vector.tensor_copy(scal[0:1, 2:3], tiT[0:1, b:b + 1])
            nc.vector.tensor_copy(scal[0:1, 3:8], misc_sb[0:1, 0:5])
            st_["scal"] = scal
            if debug:
                nc.gpsimd.dma_start(out=env["dbg_sc"].ap()[b, :, :], in_=scal[:])

            # stats matmuls: [3,512] (sum, qgx, wgx) + [1,512] (sumsq) per
            # block; sumsq via PE ones-matmul over the squared tile.
            sd = drp.tile([4, HW], F32, tag="sd", name="sd")
            for blk in range(NBLK):
                sl_ = slice(blk * BLK, (blk + 1) * BLK)
                stA = psA.tile([3, BLK], F32, tag="stA", name="stA")
                stB = psB.tile([1, BLK], F32, tag="stB", name="stB")
                for ct in range(CT):
                    sq = bigp.tile([128, BLK], F32R, tag="sq", name="sq")
                    xs = xb[ct][:, sl_]
                    nc.scalar.square(sq[:], xs[:].bitcast(F32))
                    nc.tensor.matmul(stA[:], W3[:, ct, :], xs,
                                     start=(ct == 0), stop=(ct == CT - 1))
                    nc.tensor.matmul(stB[:], onescol[:], sq[:],
                                     start=(ct == 0), stop=(ct == CT - 1))
                eva = smp.tile([3, BLK], F32, tag="eva", name="eva", bufs=2)
                evb = smp.tile([1, BLK], F32, tag="evb", name="evb", bufs=2)
                nc.scalar.mul(eva[:], stA[:], 1.0)
                nc.scalar.mul(evb[:], stB[:], 1.0)
                nc.sync.dma_start(out=sd[0:3, sl_], in_=eva[:])
                nc.sync.dma_start(out=sd[3:4, sl_], in_=evb[:])
            nc.sync.dma_start(out=nt64[b * 32:(b + 1) * 32, :, :],
                              in_=sd[:].rearrange("s (j f) -> j s f", f=128))
            return st_

        def middle_stats(sts):
            # both batches stacked on 64 partitions: halves instruction count
            # and activation-table reloads vs per-batch processing
            sc64 = smp.tile([64, 8], F32, tag="sc64", name="sc64", bufs=1)
            for b in range(BSH):
                ps32 = psM.tile([32, 8], F32, tag="psm", name=f"ps32_{b}")
                nc.tensor.matmul(ps32[:], onesrow[:, 0:32], sts[b]["scal"][:],
                                 start=True, stop=True)
                nc.vector.tensor_copy(sc64[b * 32:(b + 1) * 32, :], ps32[:])

            def st(tag):
                return smp.tile([64, 128], F32, tag=tag, name=tag, bufs=1)

            m_t = st("m_t")
            nc.scalar.mul(m_t[:], nt64[:, 0, :], 1.0 / C)
            v_t = st("v_t")
            nc.scalar.mul(v_t[:], nt64[:, 3, :], 1.0 / C)
            msq = st("msq")
            nc.vector.tensor_mul(msq[:], m_t[:], m_t[:])
            var = st("var")
            nc.vector.tensor_sub(var[:], v_t[:], msq[:])
            stdt = st("stdt")
            nc.scalar.activation(stdt[:], var[:], AF.Sqrt, bias=eps64[:], scale=1.0)
            rstd = st("rstd")
            nc.vector.reciprocal(rstd[:], stdt[:])
            t1 = st("t1")
            nc.vector.tensor_scalar_mul(t1[:], m_t[:], sc64[:, 0:1])
            nc.vector.tensor_sub(t1[:], nt64[:, 1, :], t1[:])
            nc.vector.tensor_mul(t1[:], t1[:], rstd[:])
            nc.vector.tensor_scalar_add(t1[:], t1[:], sc64[:, 1:2])
            ex_t = st("ex_t")
            zp64 = smp.tile([64, 1], F32, tag="zp64", name="zp64", bufs=1)
            nc.scalar.activation(ex_t[:], t1[:], AF.Exp, scale=SCALE, accum_out=zp64[:])
            u_t = smp.tile([64, 128], F32R, tag="u_t", name="u_t", bufs=1)
            nc.vector.tensor_mul(u_t[:], ex_t[:], rstd[:])
            t4 = st("t4")
            nc.vector.tensor_scalar_mul(t4[:], m_t[:], sc64[:, 3:4])
            nc.vector.tensor_sub(t4[:], nt64[:, 2, :], t4[:])
            nc.vector.tensor_mul(t4[:], t4[:], rstd[:])
            nc.vector.tensor_scalar_add(t4[:], t4[:], sc64[:, 4:5])
            ii = st("ii")
            iie = st("iie")
            nc.scalar.activation(iie[:], t4[:], AF.Exp, scale=-1.0)
            nc.vector.tensor_scalar_add(iie[:], iie[:], 1.0)
            nc.vector.reciprocal(ii[:], iie[:])
            den = st("den")
            nc.vector.tensor_scalar(out=den[:], in0=ii[:], scalar1=sc64[:, 2:3],
                                    scalar2=EPS_FUSE, op0=AL.add, op1=AL.add)
            rden = st("rden")
            nc.vector.reciprocal(rden[:], den[:])
            alpha = st("alpha")
            nc.vector.tensor_mul(alpha[:], ii[:], rden[:])
            beta = st("beta")
            nc.vector.tensor_scalar_mul(beta[:], rden[:], sc64[:, 2:3])
            scrk = st("scrk")
            k1p64 = smp.tile([64, 1], F32, tag="k1p64", name="k1p64", bufs=1)
            nc.vector.tensor_mul(scrk[:], u_t[:].bitcast(F32), m_t[:])
            nc.vector.reduce_sum(k1p64[:], scrk[:], axis=mybir.AxisListType.X)
            # per-batch Z, K1 via block-indicator matmul on the 64 partitions
            psz2 = psM.tile([1, 4], F32, tag="psm", name="psz2")
            nc.tensor.matmul(psz2[:, 0:2], zp64[:], bind64[:], start=True, stop=True)
            nc.tensor.matmul(psz2[:, 2:4], k1p64[:], bind64[:], start=True, stop=True)
            zk = smp.tile([1, 6], F32, tag="zk", name="zk", bufs=1)
            nc.vector.tensor_copy(zk[0:1, 0:4], psz2[:])
            nc.vector.reciprocal(zk[0:1, 4:6], zk[0:1, 0:2])
            zkr2 = smp.tile([1, 4], F32R, tag="zkr2", name="zkr2", bufs=1)
            nc.vector.tensor_copy(zkr2[0:1, 0:2], zk[0:1, 4:6])
            nc.vector.tensor_copy(zkr2[0:1, 2:4], zk[0:1, 2:4])
            ps128b = psM.tile([128, 4], F32, tag="psm", name="ps128b")
            nc.tensor.matmul(ps128b[:], onesrow[:], zkr2[:], start=True, stop=True)
            sc128b = smp.tile([128, 4], F32, tag="sc128b", name="sc128b", bufs=1)
            nc.vector.tensor_copy(sc128b[:], ps128b[:])

            # export alpha/beta/ones/u -> DRAM -> fp32r rows (per batch)
            ex4 = smp.tile([64, 4, 128], F32R, tag="ex4", name="ex4", bufs=1)
            nc.vector.tensor_copy(ex4[:, 0, :], alpha[:])
            nc.vector.tensor_copy(ex4[:, 1, :], beta[:])
            nc.vector.tensor_copy(ex4[:, 2, :], ones64x[:])
            nc.vector.tensor_copy(ex4[:, 3, :], u_t[:])
            for b in range(BSH):
                ed = drp.tile([4, HW], F32R, tag="ed", name="ed")
                for s_ in range(4):
                    nc.sync.dma_start(out=ed[s_:s_ + 1, :],
                                      in_=ex4[b * 32:(b + 1) * 32, s_, :])
                sts[b]["ed"] = ed
                if debug:
                    nc.gpsimd.dma_start(out=env["dbg_ext"].ap()[b, :, :], in_=ed[:])
            return sc128b

        def middle_tail(b, st_, sc128b):
            xb, ed = st_["xb"], st_["ed"]
            rext = smp.tile([3, HW], F32R, tag="rext", name="rext")
            nc.sync.dma_start(out=rext, in_=ed[0:3, :])
            rext_u = smp.tile([1, HW], F32R, tag="rext_u", name="rext_u", bufs=1)
            nc.sync.dma_start(out=rext_u, in_=ed[3:4, :])
            st_["rext"] = rext

            # s-contraction: s_un[c] = sum_n u[n] x[c,n]  (fused mul+reduce)
            s_acc = smp.tile([128, CT], F32, tag="s_acc", name="s_acc")
            for blk in range(NBLK):
                wb = psC.tile([128, BLK], F32, tag="conv", name="wb")
                nc.tensor.matmul(wb[:], onesrow[:], rext_u[:, blk * BLK:(blk + 1) * BLK],
                                 start=True, stop=True)
                for ct in range(CT):
                    scr = bigp.tile([128, BLK], F32, tag="scr", name="scr")
                    red = smp.tile([128, 1], F32, tag="red", name="red", bufs=3)
                    nc.vector.scalar_tensor_tensor(
                        out=scr[:], in0=xb[ct][:, blk * BLK:(blk + 1) * BLK].bitcast(F32),
                        scalar=0.0, in1=wb[:], op0=AL.bypass, op1=AL.mult,
                        accum_out=red[:])
                    if blk == 0:
                        nc.vector.tensor_copy(s_acc[:, ct:ct + 1], red[:])
                    else:
                        nc.vector.tensor_add(s_acc[:, ct:ct + 1], s_acc[:, ct:ct + 1], red[:])
            # s = g*(s_un - K1)/Z + b
            s_sb = smp.tile([128, CT], F32R, tag="s_sb", name="s_sb")
            for ct in range(CT):
                tmp = smp.tile([128, 1], F32, tag="sfin", name="sfin")
                nc.vector.tensor_scalar(out=tmp[:], in0=s_acc[:, ct:ct + 1],
                                        scalar1=sc128b[:, 2 + b:3 + b], scalar2=None,
                                        op0=AL.subtract)
                nc.vector.tensor_mul(tmp[:], tmp[:], g2[:, ct:ct + 1])
                nc.vector.tensor_scalar_mul(tmp[:], tmp[:], sc128b[:, b:b + 1])
                nc.vector.tensor_add(s_sb[:, ct:ct + 1], tmp[:], bln2[:, ct:ct + 1])
            if debug:
                nc.gpsimd.dma_start(out=env["dbg_s"].ap()[b, :, :], in_=s_sb[:])

            def matvec(wmat, rhs_sb, tag):
                out_sb = smp.tile([128, CT], F32R, tag=tag, name=tag)
                for mt in range(CT):
                    psm = psM.tile([128, 2], F32, tag="psm", name="psm")
                    for kt in range(CT):
                        nc.tensor.matmul(psm[:], wmat[:, kt, mt * 128:(mt + 1) * 128],
                                         rhs_sb[:, kt:kt + 1].to_broadcast((128, 2)),
                                         start=(kt == 0), stop=(kt == CT - 1))
                    nc.vector.tensor_copy(out_sb[:, mt:mt + 1], psm[:, 0:1])
                return out_sb

            tg_sb = matvec(wiv, s_sb, "tg_sb")
            wt_sb = matvec(wou, tg_sb, "wt_sb")
            b1_sb = matvec(wf1, wt_sb, "b1_sb")

            ext1 = smp.tile([2, CT, 128], F32R, tag="ext1", name="ext1")
            ext2 = smp.tile([4, CT, 128], F32R, tag="ext2", name="ext2")
            for mt in range(CT):
                ab1 = smp.tile([128, 2], F32R, tag="ab1", name="ab1")
                nc.vector.tensor_copy(ab1[:, 0:1], a1_sb[:, mt, b:b + 1])
                nc.vector.tensor_copy(ab1[:, 1:2], b1_sb[:, mt:mt + 1])
                pse1 = psM.tile([2, 128], F32R, tag="psm", name="pse1")
                nc.tensor.transpose(pse1[:], ab1[:], id128[:])
                nc.vector.tensor_copy(ext1[:, mt, :], pse1[:])
                ab2 = smp.tile([128, 4], F32R, tag="ab2", name="ab2")
                nc.vector.tensor_copy(ab2[:, 0:1], wv_sb[:, mt, b:b + 1])
                nc.vector.tensor_copy(ab2[:, 1:2], wt_sb[:, mt:mt + 1])
                nc.vector.tensor_copy(ab2[:, 2:3], bb2[:, mt:mt + 1])
                nc.vector.tensor_copy(ab2[:, 3:4], onescol[:])  # unused pad
                pse2 = psM.tile([4, 128], F32R, tag="psm", name="pse2")
                nc.tensor.transpose(pse2[:], ab2[:], id128[:])
                nc.vector.tensor_copy(ext2[:, mt, :], pse2[:])
            st_["ext1"], st_["ext2"] = ext1, ext2

        def pass2(b, st_):
            xb, rext, ext1, ext2 = st_["xb"], st_["rext"], st_["ext1"], st_["ext2"]
            for blk in range(NBLK):
                sl = slice(blk * BLK, (blk + 1) * BLK)
                h_ts = []
                for mt in range(CT):
                    ph = psC.tile([128, BLK], F32, tag="conv", name="ph")
                    nc.tensor.matmul(ph[:], ext1[:, mt, :], rext[0:2, sl], start=True, stop=False)
                    for kt in range(CT):
                        nc.tensor.matmul(ph[:], wf1[:, kt, mt * 128:(mt + 1) * 128],
                                         xb[kt][:, sl], start=False, stop=(kt == CT - 1))
                    h_t = bigp.tile([128, BLK], F32R, tag=f"h{mt}", name=f"h{mt}")
                    nc.scalar.activation(h_t[:], ph[:], AF.Gelu, bias=cc12[:, mt:mt + 1], scale=1.0)
                    h_ts.append(h_t)
                ot = outp.tile([128, CT, BLK], F32, tag="ot", name="ot")
                for mt in range(CT):
                    po = psC.tile([128, BLK], F32, tag="conv", name="po")
                    nc.tensor.matmul(po[:], ext2[0:3, mt, :], rext[0:3, sl], start=True, stop=False)
                    for kt in range(CT):
                        nc.tensor.matmul(po[:], wf2[:, kt, mt * 128:(mt + 1) * 128],
                                         h_ts[kt][:], start=False, stop=(kt == CT - 1))
                    nc.vector.tensor_add(ot[:, mt, :], po[:], xb[mt][:, sl])
                nc.sync.dma_start(
                    out=yout.ap()[b, :, sl].rearrange("(c p) f -> p c f", p=128),
                    in_=ot[:])

        sts = [pass1(b, xbs[b]) for b in range(BSH)]
        if STAGE <= 1:
            return
        sc128b = middle_stats(sts)
        for b in range(BSH):
            middle_tail(b, sts[b], sc128b)
        if STAGE <= 2:
            return
        for b in range(BSH):
            pass2(b, sts[b])

    for _rep in range(REPEAT):
        body()


def _prep_inputs(inputs):
    """Host-side weight preprocessing + per-core sharding + blob packing."""
    f = lambda k: np.ascontiguousarray(np.asarray(inputs[k], dtype=np.float32))
    img = f('img_feats').reshape(B, C, HW)
    txt = f('txt_feats')
    g = f('ln_img_g'); bln = f('ln_img_b')
    w_igate = f('w_igate')[0]
    v2 = lambda v: np.ascontiguousarray(v.reshape(CT, 128).T)  # [C] -> [128, 2]
    common = {
        'wf1T': np.ascontiguousarray(f('w_ffn1').T),
        'wf2T': np.ascontiguousarray(f('w_ffn2').T),
        'woutT': np.ascontiguousarray(f('w_out').T),
        'wivT': np.ascontiguousarray(f('w_img_v').T),
        'wik': f('w_img_k'),
        'wtqT': np.ascontiguousarray(f('w_txt_q').T),
        'wtvT': np.ascontiguousarray(f('w_txt_v').T),
        'g2d': v2(g),
        'bln2d': v2(bln),
        'bb2d': v2(f('b_out') + f('b_ffn2')),
        'cc12d': v2(f('w_ffn1') @ f('b_out') + f('b_ffn1')),
        'wg2d': v2(w_igate * g),
        'gt2d': f('ln_txt_g').reshape(1, GUIDE),
        'bt2d': f('ln_txt_b').reshape(1, GUIDE),
        'wtg2d': f('w_tgate').reshape(1, GUIDE),
        'btg2d': np.full((2, 1), f('b_tgate')[0], np.float32),
        'misc': np.concatenate([
            np.array([np.sum(w_igate * g), np.dot(w_igate, bln) + f('b_igate')[0]],
                     np.float32), np.zeros(6, np.float32)]).reshape(1, 8),
        'conesr': np.ones((1, 128), np.float32),
        'id128d': np.eye(128, dtype=np.float32),
        'bind64': np.repeat(np.eye(2, dtype=np.float32), 32, axis=0),
    }
    blob0 = np.empty(WTOT, np.float32)
    for name, shape in WSPEC:
        if name == 'txt':
            continue
        arr = common[name]
        assert arr.shape == shape, (name, arr.shape, shape)
        blob0[OFFS[name]:OFFS[name] + arr.size] = arr.ravel()
    in_maps = []
    for core in range(NCORES):
        sl = slice(core * BSH, (core + 1) * BSH)
        blob = blob0.copy()
        blob[OFFS['txt']:OFFS['txt'] + BSH * GUIDE] = txt[sl].ravel()
        in_maps.append({'img': np.ascontiguousarray(img[sl]),
                        'wblob': blob.reshape(1, WTOT)})
    return in_maps


def get_nc(debug=False, repeat=None):
    key = ('dbg' if debug else 'rel', repeat)
    if key not in _CACHE:
        _CACHE[key] = _build(debug, repeat)
    return _CACHE[key]


def run(inputs, debug=False):
    nc = get_nc(debug)
    in_maps = _prep_inputs(inputs)
    res = bass_utils.run_bass_kernel_spmd(nc, in_maps, core_ids=list(range(NCORES)))
    return res


def kernel(**inputs):
    res = run(inputs)
    out = np.empty((B, C, HW), np.float32)
    for core in range(NCORES):
        out[core * BSH:(core + 1) * BSH] = res.results[core]['yout']
    return out.reshape(B, C, H, W)


# revision 53
# speedup vs baseline: 1.0141x; 1.0141x over previous
# Trainium2 Bass kernel for BidirectionalCrossAttentionBlock.
#
# Key algebraic facts used (validated vs the reference to 1e-5):
#  * attn_i2t is a softmax over a size-1 axis -> identically 1.0, so
#    img_guided[b,c,n] = v_txt[b,c] (broadcast).
#  * The t2i attention, gating and out-projection collapse to rank-2
#    updates over the image:
#      out1 = x + wv (x) alpha + wt (x) beta + b_out
#    with per-position alpha[n] = img_imp/denom, beta[n] = txt_imp/denom,
#    wv = w_out@v_txt, wt = w_out@(w_img_v@s),
#    s[c] = g[c]*(sum_n u[n] x[c,n] - K1)/Z + b[c],
#    u[n] = exp(SCALE*logit[n])*rstd[n], Z = sum exp, K1 = sum u*m.
#  * logit[n] and the image gate are single matvecs against raw x plus
#    per-position corrections from the LN stats (m[n], rstd[n]).
#  * Only real heavy compute: the two FFN 1x1 convs; done as fp32r
#    matmuls (full PE rate at N=512) with the rank-2/bias terms folded
#    in as small extra-K matmuls.
#
# Sharding: pure data parallel, 2 batches per core on 8 cores.
#
# All parameters/constants are packed into a single "wblob" input tensor
# (sliced on-device with explicit access patterns): per-call argument
# handling through the PJRT tunnel costs ~11us per argument, so 2 args
# instead of 26 saves ~0.25ms per call.

import os
import numpy as np

import concourse.bacc as bacc
import concourse.tile as tile
import concourse.mybir as mybir
from concourse import bass_utils
from concourse.masks import make_identity

F32 = mybir.dt.float32
F32R = mybir.dt.float32r
AL = mybir.AluOpType
AF = mybir.ActivationFunctionType

B, C, H, W = 16, 256, 64, 64
GUIDE = 512
HW = 64 * 64  # 4096
NCORES = 8
BSH = B // NCORES  # 2 batches per core
SCALE = (C // 4) ** -0.5  # heads=4 -> 0.125
EPS_LN = 1e-5
EPS_FUSE = 1e-8
NBLK = 8          # 4096 / 512
BLK = 512
CT = 2            # channel tiles of 128
KT = GUIDE // 128  # 4

# ---- packed weight-blob layout (host packs, device slices by offset) ----
WSPEC = [
    ('wf1T', (C, C)), ('wf2T', (C, C)), ('woutT', (C, C)),
    ('wivT', (C, C)), ('wik', (C, C)),
    ('wtqT', (GUIDE, C)), ('wtvT', (GUIDE, C)),
    ('g2d', (128, CT)), ('bln2d', (128, CT)), ('bb2d', (128, CT)),
    ('cc12d', (128, CT)), ('wg2d', (128, CT)),
    ('gt2d', (1, GUIDE)), ('bt2d', (1, GUIDE)), ('wtg2d', (1, GUIDE)),
    ('btg2d', (2, 1)), ('misc', (1, 8)),
    ('conesr', (1, 128)), ('id128d', (128, 128)),
    ('bind64', (64, 2)),
    ('txt', (BSH, GUIDE)),
]
OFFS = {}
_off = 0
for _n, _s in WSPEC:
    OFFS[_n] = _off
    _off += int(np.prod(_s))
WTOT = _off

_CACHE = {}


def _build(debug=False, repeat=None):
    nc = bacc.Bacc("TRN2", target_bir_lowering=False, debug=False)

    img = nc.dram_tensor("img", [BSH, C, HW], F32R, kind="ExternalInput")
    wblob = nc.dram_tensor("wblob", [1, WTOT], F32, kind="ExternalInput")
    yout = nc.dram_tensor("yout", [BSH, C, HW], F32, kind="ExternalOutput")
    if debug:
        dbg_ext = nc.dram_tensor("dbg_ext", [BSH, 4, HW], F32, kind="ExternalOutput")
        dbg_s = nc.dram_tensor("dbg_s", [BSH, 128, CT], F32, kind="ExternalOutput")
        dbg_sc = nc.dram_tensor("dbg_sc", [BSH, 1, 8], F32, kind="ExternalOutput")

    env = {k: v for k, v in locals().items()}
    from contextlib import ExitStack
    with tile.TileContext(nc) as tc, ExitStack() as ctx:
        env["wp"] = ctx.enter_context(tc.tile_pool(name="wp", bufs=1))
        env["xp"] = ctx.enter_context(tc.tile_pool(name="xp", bufs=2))
        env["bigp"] = ctx.enter_context(tc.tile_pool(name="bigp", bufs=2))
        env["outp"] = ctx.enter_context(tc.tile_pool(name="outp", bufs=2))
        env["smp"] = ctx.enter_context(tc.tile_pool(name="smp", bufs=2))
        env["drp"] = ctx.enter_context(tc.tile_pool(name="drp", bufs=2, space="DRAM"))
        env["psC"] = ctx.enter_context(tc.tile_pool(name="psC", bufs=3, space="PSUM"))
        env["psA"] = ctx.enter_context(tc.tile_pool(name="psA", bufs=2, space="PSUM"))
        env["psB"] = ctx.enter_context(tc.tile_pool(name="psB", bufs=2, space="PSUM"))
        env["psM"] = ctx.enter_context(tc.tile_pool(name="psM", bufs=1, space="PSUM"))
        _emit(nc, tc, env, debug, repeat)
    nc.compile()
    return nc


def _emit(nc, tc, env, debug, repeat=None):
    STAGE = float(os.environ.get("KSTAGE", "9"))
    REPEAT = int(os.environ.get("KREPEAT", "1")) if repeat is None else repeat
    wp, xp, bigp, outp, smp, drp = (env[k] for k in ("wp", "xp", "bigp", "outp", "smp", "drp"))
    psC, psA, psB, psM = (env[k] for k in ("psC", "psA", "psB", "psM"))
    img, yout = env["img"], env["yout"]

    base = env["wblob"].ap()

    def wap(name, dims, off=0, dt_=None):
        # dims: list of [stride, count]; offsets/strides in elements
        a = base.__class__(tensor=base.tensor, offset=OFFS[name] + off,
                           ap=[list(d) for d in dims])
        if dt_ is not None:
            a = a.bitcast(dt_)
        return a

    def body():
        # ---------------- load weights/consts from blob ----------------
        # Consolidated: the blob layout is contiguous, so the five [C,C]
        # fp32r matrices land in ONE DMA as [128, 10, C] (128-row chunks),
        # the two [GUIDE,C] in one [128, 8, C], the five [128,CT] vectors in
        # one [128, 5, CT], the three [1,GUIDE] rows in one broadcast
        # [2, 3, GUIDE]. Each dma_start costs ~565ns of SP sequencer time,
        # so ~30 -> ~11 descriptors-config saves ~11us at the head.
        W5 = wp.tile([128, 10, C], F32R, tag="W5")
        nc.sync.dma_start(out=W5, in_=wap("wf1T", [[C, 128], [128 * C, 10], [1, C]],
                                          dt_=F32R))
        wf1, wf2 = W5[:, 0:2, :], W5[:, 2:4, :]
        wou, wiv, wik_s = W5[:, 4:6, :], W5[:, 6:8, :], W5[:, 8:10, :]
        W8 = wp.tile([128, 8, C], F32R, tag="W8")
        nc.sync.dma_start(out=W8, in_=wap("wtqT", [[C, 128], [128 * C, 8], [1, C]],
                                          dt_=F32R))
        wtq, wtv = W8[:, 0:4, :], W8[:, 4:8, :]

        V5 = wp.tile([128, 5, CT], F32, tag="V5")
        nc.sync.dma_start(out=V5, in_=wap("g2d", [[CT, 128], [128 * CT, 5], [1, CT]]))
        g2, bln2, bb2, cc12, wg2 = (V5[:, i, :] for i in range(5))

        R3 = wp.tile([2, 3, GUIDE], F32, tag="R3")
        nc.sync.dma_start(out=R3, in_=wap("gt2d", [[0, 2], [GUIDE, 3], [1, GUIDE]]))
        gt2, bt2, wtg2 = R3[:, 0, :], R3[:, 1, :], R3[:, 2, :]
        btg2 = wp.tile([2, 1], F32, tag="btg2d")
        nc.sync.dma_start(out=btg2, in_=wap("btg2d", [[1, 2], [1, 1]]))
        misc_sb = wp.tile([1, 8], F32, tag="misc")
        nc.sync.dma_start(out=misc_sb, in_=wap("misc", [[8, 1], [1, 8]]))

        onesrow = wp.tile([1, 128], F32R, tag="onesrow")
        nc.sync.dma_start(out=onesrow, in_=wap("conesr", [[128, 1], [1, 128]], dt_=F32R))
        onescol = wp.tile([128, 1], F32R, tag="onescol")
        nc.sync.dma_start(out=onescol, in_=wap("conesr", [[1, 128], [1, 1]], dt_=F32R))
        ones64x = wp.tile([64, 128], F32R, tag="ones64x")
        nc.sync.dma_start(out=ones64x, in_=wap("conesr", [[0, 64], [1, 128]], dt_=F32R))
        id128 = wp.tile([128, 128], F32R, tag="id128")
        nc.sync.dma_start(out=id128, in_=wap("id128d", [[128, 128], [1, 128]], dt_=F32R))
        bind64 = wp.tile([64, 2], F32, tag="bind64")
        nc.sync.dma_start(out=bind64, in_=wap("bind64", [[2, 64], [1, 2]]))
        id2 = wp.tile([2, 2], F32, tag="id2")
        make_identity(nc, id2[:])
        eps2 = wp.tile([2, 1], F32, tag="eps2")
        nc.vector.memset(eps2[:], EPS_LN)
        eps64 = wp.tile([64, 1], F32, tag="eps64")
        nc.vector.memset(eps64[:], EPS_LN)

        if STAGE <= 0:
            return
        # txt load first (head of the preamble dependency chain), then
        # prefetch img tiles for both batches so the bulk input DMA overlaps
        # the txt-side preamble compute
        txt_sb = smp.tile([2, GUIDE], F32, tag="txt", bufs=1)
        nc.sync.dma_start(out=txt_sb, in_=wap("txt", [[GUIDE, 2], [1, GUIDE]]))
        xbs = []
        for b in range(BSH):
            xb_ = []
            for ct in range(CT):
                t = xp.tile([128, HW], F32R, tag=f"x{b}_{ct}", name=f"x{b}_{ct}", bufs=1)
                nc.sync.dma_start(out=t, in_=img.ap()[b, ct * 128:(ct + 1) * 128, :])
                xb_.append(t)
            xbs.append(xb_)
        nt64 = smp.tile([64, 4, 128], F32, tag="nt64", name="nt64", bufs=1)
        bnst = smp.tile([2, 6], F32, tag="bnst")
        nc.vector.bn_stats(out=bnst[:], in_=txt_sb[:])
        mv = smp.tile([2, 2], F32, tag="mv")
        nc.vector.bn_aggr(out=mv[:], in_=bnst[:])
        tstd = smp.tile([2, 1], F32, tag="tstd")
        nc.scalar.activation(tstd[:], mv[:, 1:2], AF.Sqrt, bias=eps2[:], scale=1.0)
        trstd = smp.tile([2, 1], F32, tag="trstd")
        nc.vector.reciprocal(trstd[:], tstd[:])
        txtn = smp.tile([2, GUIDE], F32, tag="txtn", bufs=1)
        nc.vector.tensor_scalar(out=txtn[:], in0=txt_sb[:], scalar1=mv[:, 0:1],
                                scalar2=trstd[:], op0=AL.subtract, op1=AL.mult)
        nc.vector.tensor_mul(txtn[:], txtn[:], gt2[:])
        nc.vector.tensor_add(txtn[:], txtn[:], bt2[:])
        if STAGE <= 0.2:
            return
        # txt_imp = sigmoid(txtn @ w_tgate + b_tgate)  [2,1]
        scr2 = smp.tile([2, GUIDE], F32, tag="scr2", bufs=1)
        tip = smp.tile([2, 1], F32, tag="tip")
        nc.vector.tensor_mul(scr2[:], txtn[:], wtg2[:])
        nc.vector.reduce_sum(tip[:], scr2[:], axis=mybir.AxisListType.X)
        nc.vector.tensor_add(tip[:], tip[:], btg2[:])
        # sigmoid via exp: keeps Act on the exp table set (no table reload)
        ti = smp.tile([2, 1], F32, tag="ti")
        tie = smp.tile([2, 1], F32, tag="tie")
        nc.scalar.activation(tie[:], tip[:], AF.Exp, scale=-1.0)
        nc.vector.tensor_scalar_add(tie[:], tie[:], 1.0)
        nc.vector.reciprocal(ti[:], tie[:])
        if STAGE <= 0.4:
            return
        # transpose ti -> [1, 2]
        pst = psM.tile([1, 2], F32, tag="psm")
        nc.tensor.transpose(pst[:], ti[:], id2[:])
        tiT = smp.tile([1, 2], F32, tag="tiT")
        nc.vector.tensor_copy(tiT[:], pst[:])
        if STAGE <= 0.5:
            return
        # txtn -> transposed fp32r [128, 4, 2]
        txtnT = smp.tile([128, KT, 2], F32R, tag="txtnT")
        for kt in range(KT):
            ps2 = psM.tile([128, 2], F32, tag="psm")
            nc.tensor.transpose(ps2[:], txtn[:, kt * 128:(kt + 1) * 128], id2[:])
            nc.vector.tensor_copy(txtnT[:, kt, :], ps2[:])
        if STAGE <= 0.6:
            return
        # q = w_txt_q @ txt_n, v = w_txt_v @ txt_n   -> [128, mt, b] fp32r
        q_sb = smp.tile([128, CT, 2], F32R, tag="q_sb")
        v_sb = smp.tile([128, CT, 2], F32R, tag="v_sb")
        for (wmat, dst) in ((wtq, q_sb), (wtv, v_sb)):
            for mt in range(CT):
                psq = psM.tile([128, 2], F32, tag="psm")
                for kt in range(KT):
                    nc.tensor.matmul(psq[:], wmat[:, kt, mt * 128:(mt + 1) * 128],
                                     txtnT[:, kt, :], start=(kt == 0), stop=(kt == KT - 1))
                nc.vector.tensor_copy(dst[:, mt, :], psq[:])
        if STAGE <= 0.7:
            return
        # qk = w_img_k.T @ q  -> [128, mt, b] (fp32 copy for DVE use)
        qk_sb = smp.tile([128, CT, 2], F32, tag="qk_sb")
        for mt in range(CT):
            psk = psM.tile([128, 2], F32, tag="psm")
            for kt in range(CT):
                nc.tensor.matmul(psk[:], wik_s[:, kt, mt * 128:(mt + 1) * 128],
                                 q_sb[:, kt, :], start=(kt == 0), stop=(kt == CT - 1))
            nc.vector.tensor_copy(qk_sb[:, mt, :], psk[:])
        if STAGE <= 0.8:
            return
        # wv = w_out @ v -> [128, mt, b] fp32r ; A1 = w_ffn1 @ wv
        wv_sb = smp.tile([128, CT, 2], F32R, tag="wv_sb")
        for mt in range(CT):
            psv = psM.tile([128, 2], F32, tag="psm")
            for kt in range(CT):
                nc.tensor.matmul(psv[:], wou[:, kt, mt * 128:(mt + 1) * 128],
                                 v_sb[:, kt, :], start=(kt == 0), stop=(kt == CT - 1))
            nc.vector.tensor_copy(wv_sb[:, mt, :], psv[:])
        a1_sb = smp.tile([128, CT, 2], F32R, tag="a1_sb")
        for mt in range(CT):
            psa = psM.tile([128, 2], F32, tag="psm")
            for kt in range(CT):
                nc.tensor.matmul(psa[:], wf1[:, kt, mt * 128:(mt + 1) * 128],
                                 wv_sb[:, kt, :], start=(kt == 0), stop=(kt == CT - 1))
            nc.vector.tensor_copy(a1_sb[:, mt, :], psa[:])

        # ---------------- per-batch, phase-major ----------------
        def pass1(b, xb):
            st_ = {}
            st_["xb"] = xb
            # W3 lhsT = [ones, qg_b, wg]
            W3 = smp.tile([128, CT, 3], F32R, tag="W3", name="W3")
            for ct in range(CT):
                nc.vector.tensor_copy(W3[:, ct, 0:1], onescol[:])
                nc.vector.tensor_mul(W3[:, ct, 1:2], qk_sb[:, ct, b:b + 1], g2[:, ct:ct + 1])
                nc.vector.tensor_copy(W3[:, ct, 2:3], wg2[:, ct:ct + 1])
            # Sq = sum qg ; Cq = sum qk*b_ln
            ps1 = psM.tile([1, 2], F32, tag="psm", name="ps1")
            for ct in range(CT):
                nc.tensor.matmul(ps1[:], W3[:, ct, 1:2], onescol[:].to_broadcast((128, 2)),
                                 start=(ct == 0), stop=(ct == CT - 1))
            tcq = smp.tile([128, CT], F32R, tag="tcq", name="tcq")
            for ct in range(CT):
                nc.vector.tensor_mul(tcq[:, ct:ct + 1], qk_sb[:, ct, b:b + 1], bln2[:, ct:ct + 1])
            ps1b = psM.tile([1, 2], F32, tag="psm", name="ps1b")
            for ct in range(CT):
                nc.tensor.matmul(ps1b[:], tcq[:, ct:ct + 1], onescol[:].to_broadcast((128, 2)),
                                 start=(ct == 0), stop=(ct == CT - 1))
            scal = smp.tile([1, 8], F32R, tag="scal", name="scal")
            nc.vector.tensor_copy(scal[0:1, 0:1], ps1[:, 0:1])
            nc.vector.tensor_copy(scal[0:1, 1:2], ps1b[:, 0:1])
            nc.vector.tensor_copy(scal[0:1, 2:3], tiT[0:1, b:b + 1])
            nc.vector.tensor_copy(scal[0:1, 3:8], misc_sb[0:1, 0:5])
            st_["scal"] = scal
            if debug:
                nc.gpsimd.dma_start(out=env["dbg_sc"].ap()[b, :, :], in_=scal[:])

            # stats matmuls: [3,512] (sum, qgx, wgx) + [1,512] (sumsq) per
            # block; sumsq via PE ones-matmul over the squared tile.
            sd = drp.tile([4, HW], F32, tag="sd", name="sd")
            for blk in range(NBLK):
                sl_ = slice(blk * BLK, (blk + 1) * BLK)
                stA = psA.tile([3, BLK], F32, tag="stA", name="stA")
                stB = psB.tile([1, BLK], F32, tag="stB", name="stB")
                for ct in range(CT):
                    sq = bigp.tile([128, BLK], F32R, tag="sq", name="sq")
                    xs = xb[ct][:, sl_]
                    nc.scalar.square(sq[:], xs[:].bitcast(F32))
                    nc.tensor.matmul(stA[:], W3[:, ct, :], xs,
                                     start=(ct == 0), stop=(ct == CT - 1))
                    nc.tensor.matmul(stB[:], onescol[:], sq[:],
                                     start=(ct == 0), stop=(ct == CT - 1))
                eva = smp.tile([3, BLK], F32, tag="eva", name="eva", bufs=2)
                evb = smp.tile([1, BLK], F32, tag="evb", name="evb", bufs=2)
                nc.scalar.mul(eva[:], stA[:], 1.0)
                nc.scalar.mul(evb[:], stB[:], 1.0)
                nc.scalar.dma_start(out=sd[0:3, sl_], in_=eva[:])
                nc.scalar.dma_start(out=sd[3:4, sl_], in_=evb[:])
            nc.scalar.dma_start(out=nt64[b * 32:(b + 1) * 32, :, :],
                                in_=sd[:].rearrange("s (j f) -> j s f", f=128))
            return st_

        def middle_stats(sts):
            # both batches stacked on 64 partitions: halves instruction count
            # and activation-table reloads vs per-batch processing
            sc64 = smp.tile([64, 8], F32, tag="sc64", name="sc64", bufs=1)
            for b in range(BSH):
                ps32 = psM.tile([32, 8], F32, tag="psm", name=f"ps32_{b}")
                nc.tensor.matmul(ps32[:], onesrow[:, 0:32], sts[b]["scal"][:],
                                 start=True, stop=True)
                nc.vector.tensor_copy(sc64[b * 32:(b + 1) * 32, :], ps32[:])

            def st(tag):
                return smp.tile([64, 128], F32, tag=tag, name=tag, bufs=1)

            m_t = st("m_t")
            nc.scalar.mul(m_t[:], nt64[:, 0, :], 1.0 / C)
            v_t = st("v_t")
            nc.scalar.mul(v_t[:], nt64[:, 3, :], 1.0 / C)
            msq = st("msq")
            nc.vector.tensor_mul(msq[:], m_t[:], m_t[:])
            var = st("var")
            nc.vector.tensor_sub(var[:], v_t[:], msq[:])
            stdt = st("stdt")
            nc.scalar.activation(stdt[:], var[:], AF.Sqrt, bias=eps64[:], scale=1.0)
            rstd = st("rstd")
            nc.vector.reciprocal(rstd[:], stdt[:])
            t1 = st("t1")
            nc.vector.tensor_scalar_mul(t1[:], m_t[:], sc64[:, 0:1])
            nc.vector.tensor_sub(t1[:], nt64[:, 1, :], t1[:])
            nc.vector.tensor_mul(t1[:], t1[:], rstd[:])
            nc.vector.tensor_scalar_add(t1[:], t1[:], sc64[:, 1:2])
            ex_t = st("ex_t")
            zp64 = smp.tile([64, 1], F32, tag="zp64", name="zp64", bufs=1)
            nc.scalar.activation(ex_t[:], t1[:], AF.Exp, scale=SCALE, accum_out=zp64[:])
            u_t = smp.tile([64, 128], F32R, tag="u_t", name="u_t", bufs=1)
            nc.vector.tensor_mul(u_t[:], ex_t[:], rstd[:])
            t4 = st("t4")
            nc.vector.tensor_scalar_mul(t4[:], m_t[:], sc64[:, 3:4])
            nc.vector.tensor_sub(t4[:], nt64[:, 2, :], t4[:])
            nc.vector.tensor_mul(t4[:], t4[:], rstd[:])
            nc.vector.tensor_scalar_add(t4[:], t4[:], sc64[:, 4:5])
            ii = st("ii")
            iie = st("iie")
            nc.scalar.activation(iie[:], t4[:], AF.Exp, scale=-1.0)
            nc.vector.tensor_scalar_add(iie[:], iie[:], 1.0)
            nc.vector.reciprocal(ii[:], iie[:])
            den = st("den")
            nc.vector.tensor_scalar(out=den[:], in0=ii[:], scalar1=sc64[:, 2:3],
                                    scalar2=EPS_FUSE, op0=AL.add, op1=AL.add)
            rden = st("rden")
            nc.vector.reciprocal(rden[:], den[:])
            alpha = st("alpha")
            nc.vector.tensor_mul(alpha[:], ii[:], rden[:])
            beta = st("beta")
            nc.vector.tensor_scalar_mul(beta[:], rden[:], sc64[:, 2:3])
            scrk = st("scrk")
            k1p64 = smp.tile([64, 1], F32, tag="k1p64", name="k1p64", bufs=1)
            nc.vector.tensor_mul(scrk[:], u_t[:].bitcast(F32), m_t[:])
            nc.vector.reduce_sum(k1p64[:], scrk[:], axis=mybir.AxisListType.X)
            # per-batch Z, K1 via block-indicator matmul on the 64 partitions
            psz2 = psM.tile([1, 4], F32, tag="psm", name="psz2")
            nc.tensor.matmul(psz2[:, 0:2], zp64[:], bind64[:], start=True, stop=True)
            nc.tensor.matmul(psz2[:, 2:4], k1p64[:], bind64[:], start=True, stop=True)
            zk = smp.tile([1, 6], F32, tag="zk", name="zk", bufs=1)
            nc.vector.tensor_copy(zk[0:1, 0:4], psz2[:])
            nc.vector.reciprocal(zk[0:1, 4:6], zk[0:1, 0:2])
            zkr2 = smp.tile([1, 4], F32R, tag="zkr2", name="zkr2", bufs=1)
            nc.vector.tensor_copy(zkr2[0:1, 0:2], zk[0:1, 4:6])
            nc.vector.tensor_copy(zkr2[0:1, 2:4], zk[0:1, 2:4])
            ps128b = psM.tile([128, 4], F32, tag="psm", name="ps128b")
            nc.tensor.matmul(ps128b[:], onesrow[:], zkr2[:], start=True, stop=True)
            sc128b = smp.tile([128, 4], F32, tag="sc128b", name="sc128b", bufs=1)
            nc.vector.tensor_copy(sc128b[:], ps128b[:])

            # export alpha/beta/ones/u -> DRAM -> fp32r rows (per batch)
            ex4 = smp.tile([64, 4, 128], F32R, tag="ex4", name="ex4", bufs=1)
            nc.vector.tensor_copy(ex4[:, 0, :], alpha[:])
            nc.vector.tensor_copy(ex4[:, 1, :], beta[:])
            nc.vector.tensor_copy(ex4[:, 2, :], ones64x[:])
            nc.vector.tensor_copy(ex4[:, 3, :], u_t[:])
            for b in range(BSH):
                ed = drp.tile([4, HW], F32R, tag="ed", name="ed")
                for s_ in range(4):
                    nc.scalar.dma_start(out=ed[s_:s_ + 1, :],
                                        in_=ex4[b * 32:(b + 1) * 32, s_, :])
                sts[b]["ed"] = ed
                if debug:
                    nc.gpsimd.dma_start(out=env["dbg_ext"].ap()[b, :, :], in_=ed[:])
            return sc128b

        def middle_tail(b, st_, sc128b):
            xb, ed = st_["xb"], st_["ed"]
            rext = smp.tile([3, HW], F32R, tag="rext", name="rext")
            nc.scalar.dma_start(out=rext, in_=ed[0:3, :])
            rext_u = smp.tile([1, HW], F32R, tag="rext_u", name="rext_u", bufs=1)
            nc.scalar.dma_start(out=rext_u, in_=ed[3:4, :])
            st_["rext"] = rext

            # s-contraction: s_un[c] = sum_n u[n] x[c,n]  (fused mul+reduce)
            s_acc = smp.tile([128, CT], F32, tag="s_acc", name="s_acc")
            for blk in range(NBLK):
                wb = psC.tile([128, BLK], F32, tag="conv", name="wb")
                nc.tensor.matmul(wb[:], onesrow[:], rext_u[:, blk * BLK:(blk + 1) * BLK],
                                 start=True, stop=True)
                for ct in range(CT):
                    scr = bigp.tile([128, BLK], F32, tag="scr", name="scr")
                    red = smp.tile([128, 1], F32, tag="red", name="red", bufs=3)
                    nc.vector.scalar_tensor_tensor(
                        out=scr[:], in0=xb[ct][:, blk * BLK:(blk + 1) * BLK].bitcast(F32),
                        scalar=0.0, in1=wb[:], op0=AL.bypass, op1=AL.mult,
                        accum_out=red[:])
                    if blk == 0:
                        nc.vector.tensor_copy(s_acc[:, ct:ct + 1], red[:])
                    else:
                        nc.vector.tensor_add(s_acc[:, ct:ct + 1], s_acc[:, ct:ct + 1], red[:])
            # s = g*(s_un - K1)/Z + b
            s_sb = smp.tile([128, CT], F32R, tag="s_sb", name="s_sb")
            for ct in range(CT):
                tmp = smp.tile([128, 1], F32, tag="sfin", name="sfin")
                nc.vector.tensor_scalar(out=tmp[:], in0=s_acc[:, ct:ct + 1],
                                        scalar1=sc128b[:, 2 + b:3 + b], scalar2=None,
                                        op0=AL.subtract)
                nc.vector.tensor_mul(tmp[:], tmp[:], g2[:, ct:ct + 1])
                nc.vector.tensor_scalar_mul(tmp[:], tmp[:], sc128b[:, b:b + 1])
                nc.vector.tensor_add(s_sb[:, ct:ct + 1], tmp[:], bln2[:, ct:ct + 1])
            if debug:
                nc.gpsimd.dma_start(out=env["dbg_s"].ap()[b, :, :], in_=s_sb[:])

            def matvec(wmat, rhs_sb, tag):
                out_sb = smp.tile([128, CT], F32R, tag=tag, name=tag)
                for mt in range(CT):
                    psm = psM.tile([128, 2], F32, tag="psm", name="psm")
                    for kt in range(CT):
                        nc.tensor.matmul(psm[:], wmat[:, kt, mt * 128:(mt + 1) * 128],
                                         rhs_sb[:, kt:kt + 1].to_broadcast((128, 2)),
                                         start=(kt == 0), stop=(kt == CT - 1))
                    nc.vector.tensor_copy(out_sb[:, mt:mt + 1], psm[:, 0:1])
                return out_sb

            tg_sb = matvec(wiv, s_sb, "tg_sb")
            wt_sb = matvec(wou, tg_sb, "wt_sb")
            b1_sb = matvec(wf1, wt_sb, "b1_sb")

            ext1 = smp.tile([2, CT, 128], F32R, tag="ext1", name="ext1")
            ext2 = smp.tile([4, CT, 128], F32R, tag="ext2", name="ext2")
            for mt in range(CT):
                ab1 = smp.tile([128, 2], F32R, tag="ab1", name="ab1")
                nc.vector.tensor_copy(ab1[:, 0:1], a1_sb[:, mt, b:b + 1])
                nc.vector.tensor_copy(ab1[:, 1:2], b1_sb[:, mt:mt + 1])
                pse1 = psM.tile([2, 128], F32R, tag="psm", name="pse1")
                nc.tensor.transpose(pse1[:], ab1[:], id128[:])
                nc.vector.tensor_copy(ext1[:, mt, :], pse1[:])
                ab2 = smp.tile([128, 4], F32R, tag="ab2", name="ab2")
                nc.vector.tensor_copy(ab2[:, 0:1], wv_sb[:, mt, b:b + 1])
                nc.vector.tensor_copy(ab2[:, 1:2], wt_sb[:, mt:mt + 1])
                nc.vector.tensor_copy(ab2[:, 2:3], bb2[:, mt:mt + 1])
                nc.vector.tensor_copy(ab2[:, 3:4], onescol[:])  # unused pad
                pse2 = psM.tile([4, 128], F32R, tag="psm", name="pse2")
                nc.tensor.transpose(pse2[:], ab2[:], id128[:])
                nc.vector.tensor_copy(ext2[:, mt, :], pse2[:])
            st_["ext1"], st_["ext2"] = ext1, ext2

        def pass2(b, st_):
            xb, rext, ext1, ext2 = st_["xb"], st_["rext"], st_["ext1"], st_["ext2"]
            for blk in range(NBLK):
                sl = slice(blk * BLK, (blk + 1) * BLK)
                h_ts = []
                for mt in range(CT):
                    ph = psC.tile([128, BLK], F32, tag="conv", name="ph")
                    nc.tensor.matmul(ph[:], ext1[:, mt, :], rext[0:2, sl], start=True, stop=False)
                    for kt in range(CT):
                        nc.tensor.matmul(ph[:], wf1[:, kt, mt * 128:(mt + 1) * 128],
                                         xb[kt][:, sl], start=False, stop=(kt == CT - 1))
                    h_t = bigp.tile([128, BLK], F32R, tag=f"h{mt}", name=f"h{mt}")
                    nc.scalar.activation(h_t[:], ph[:], AF.Gelu, bias=cc12[:, mt:mt + 1], scale=1.0)
                    h_ts.append(h_t)
                ot = outp.tile([128, CT, BLK], F32, tag="ot", name="ot")
                for mt in range(CT):
                    po = psC.tile([128, BLK], F32, tag="conv", name="po")
                    nc.tensor.matmul(po[:], ext2[0:3, mt, :], rext[0:3, sl], start=True, stop=False)
                    for kt in range(CT):
                        nc.tensor.matmul(po[:], wf2[:, kt, mt * 128:(mt + 1) * 128],
                                         h_ts[kt][:], start=False, stop=(kt == CT - 1))
                    nc.vector.tensor_add(ot[:, mt, :], po[:], xb[mt][:, sl])
                nc.sync.dma_start(
                    out=yout.ap()[b, :, sl].rearrange("(c p) f -> p c f", p=128),
                    in_=ot[:])

        sts = [pass1(b, xbs[b]) for b in range(BSH)]
        if STAGE <= 1:
            return
        sc128b = middle_stats(sts)
        for b in range(BSH):
            middle_tail(b, sts[b], sc128b)
        if STAGE <= 2:
            return
        for b in range(BSH):
            pass2(b, sts[b])

    for _rep in range(REPEAT):
        body()


def _prep_inputs(inputs):
    """Host-side weight preprocessing + per-core sharding + blob packing."""
    f = lambda k: np.ascontiguousarray(np.asarray(inputs[k], dtype=np.float32))
    img = f('img_feats').reshape(B, C, HW)
    txt = f('txt_feats')
    g = f('ln_img_g'); bln = f('ln_img_b')
    w_igate = f('w_igate')[0]
    v2 = lambda v: np.ascontiguousarray(v.reshape(CT, 128).T)  # [C] -> [128, 2]
    common = {
        'wf1T': np.ascontiguousarray(f('w_ffn1').T),
        'wf2T': np.ascontiguousarray(f('w_ffn2').T),
        'woutT': np.ascontiguousarray(f('w_out').T),
        'wivT': np.ascontiguousarray(f('w_img_v').T),
        'wik': f('w_img_k'),
        'wtqT': np.ascontiguousarray(f('w_txt_q').T),
        'wtvT': np.ascontiguousarray(f('w_txt_v').T),
        'g2d': v2(g),
        'bln2d': v2(bln),
        'bb2d': v2(f('b_out') + f('b_ffn2')),
        'cc12d': v2(f('w_ffn1') @ f('b_out') + f('b_ffn1')),
        'wg2d': v2(w_igate * g),
        'gt2d': f('ln_txt_g').reshape(1, GUIDE),
        'bt2d': f('ln_txt_b').reshape(1, GUIDE),
        'wtg2d': f('w_tgate').reshape(1, GUIDE),
        'btg2d': np.full((2, 1), f('b_tgate')[0], np.float32),
        'misc': np.concatenate([
            np.array([np.sum(w_igate * g), np.dot(w_igate, bln) + f('b_igate')[0]],
                     np.float32), np.zeros(6, np.float32)]).reshape(1, 8),
        'conesr': np.ones((1, 128), np.float32),
        'id128d': np.eye(128, dtype=np.float32),
        'bind64': np.repeat(np.eye(2, dtype=np.float32), 32, axis=0),
    }
    blob0 = np.empty(WTOT, np.float32)
    for name, shape in WSPEC:
        if name == 'txt':
            continue
        arr = common[name]
        assert arr.shape == shape, (name, arr.shape, shape)
        blob0[OFFS[name]:OFFS[name] + arr.size] = arr.ravel()
    in_maps = []
    for core in range(NCORES):
        sl = slice(core * BSH, (core + 1) * BSH)
        blob = blob0.copy()
        blob[OFFS['txt']:OFFS['txt'] + BSH * GUIDE] = txt[sl].ravel()
        in_maps.append({'img': np.ascontiguousarray(img[sl]),
                        'wblob': blob.reshape(1, WTOT)})
    return in_maps


def get_nc(debug=False, repeat=None):
    key = ('dbg' if debug else 'rel', repeat)
    if key not in _CACHE:
        _CACHE[key] = _build(debug, repeat)
    return _CACHE[key]


def run(inputs, debug=False):
    nc = get_nc(debug)
    in_maps = _prep_inputs(inputs)
    res = bass_utils.run_bass_kernel_spmd(nc, in_maps, core_ids=list(range(NCORES)))
    return res


def kernel(**inputs):
    res = run(inputs)
    out = np.empty((B, C, HW), np.float32)
    for core in range(NCORES):
        out[core * BSH:(core + 1) * BSH] = res.results[core]['yout']
    return out.reshape(B, C, H, W)
